# revision 1
# baseline (speedup 1.0000x reference)
"""Trainium2 Bass kernel for nn_Net_64046552318449.

Sequential RNN scan: h_{t+1} = sigmoid(x_t*w_in + (2h_t-1) @ W_eff.T + b_h),
SEQ=2048 steps, BATCH=128, HID=256. Outputs y[:, :, 0] and the full hidden
trajectory out (batch, seq, hid).

Strategy:
  - Data-parallel over batch: 8 cores x 16 batch each. No collectives.
  - Recurrence layout "formulation B": psum pre.T laid out as
    (hid-chunk on partitions, batch on free dim).  The activation
    gT = tanh(pre/2) = 2*sigmoid(pre)-1 lands in SBUF in exactly the
    layout the next step's matmul rhs needs -> zero transposes.
  - Per step: 4 bf16 matmuls (2 j-chunks x 2 k-chunks of W_eff.T) accumulate
    into per-step 16-col slices of 512-col psum banks that were pre-seeded
    with the x_t*w_in + b_h term by one fat K=2 matmul per 64-step block.
  - Banks alternate by step parity so the tensor engine never writes a bank
    that ScalarE/VectorE are still reading (fatal psum collision otherwise).
  - Device ships raw fp32 `pre`; sigmoid and the final y projection happen
    host-side in float64, so output precision is limited only by the bf16
    trajectory (l2 rel err ~4e-4).
"""

import numpy as np

SEQ, BATCH, HID, OUT = 2048, 128, 256, 2
NCORES = 8
BSH = BATCH // NCORES        # batch shard per core = 16
BLK = 64                     # psum block: 64 steps (32 even + 32 odd slices)
NBLK = SEQ // BLK            # 32
OBLK = 32                    # output staging block steps
NOBLK = SEQ // OBLK          # 64

_CACHE = {}


def _build_program():
    import contextlib
    import concourse.bass as bass
    import concourse.mybir as mybir

    dt = mybir.dt
    Tanh = mybir.ActivationFunctionType.Tanh

    nc = bass.Bass(trn_type="TRN2", target_bir_lowering=False)
    st = contextlib.ExitStack()

    xext_d = nc.dram_tensor("xext", (2, SEQ * BSH), dt.float32, kind="ExternalInput")
    wsb_d = nc.dram_tensor("wsb", (128, 512), dt.bfloat16, kind="ExternalInput")
    wb_d = nc.dram_tensor("wb", (2, 256), dt.float32, kind="ExternalInput")
    g0_d = nc.dram_tensor("g0", (128, 2 * BSH), dt.bfloat16, kind="ExternalInput")
    out_d = nc.dram_tensor(
        "pre_out", (NOBLK, 128, OBLK * 2 * BSH), dt.float32, kind="ExternalOutput"
    )

    xext = st.enter_context(nc.sbuf_tensor([2, SEQ * BSH], dt.float32))
    wsb = st.enter_context(nc.sbuf_tensor([128, 512], dt.bfloat16))
    wb = st.enter_context(nc.sbuf_tensor([2, 256], dt.float32))
    g0 = st.enter_context(nc.sbuf_tensor([128, 2 * BSH], dt.bfloat16))
    # gT ring: 2 slots of (128, 32)
    gT = st.enter_context(nc.sbuf_tensor([128, 2 * 2 * BSH], dt.bfloat16))
    # output staging: 2 slots of (128, OBLK*32)
    stage = st.enter_context(nc.sbuf_tensor([128, 2 * OBLK * 2 * BSH], dt.float32))
    # two psum groups of 4 banks: [E_j0 | E_j1 | O_j0 | O_j1], 512 cols each
    psA = st.enter_context(nc.psum_tensor([128, 2048], dt.float32))
    psB = st.enter_context(nc.psum_tensor([128, 2048], dt.float32))

    s_in = st.enter_context(nc.semaphore(name="s_in"))
    s_pe = st.enter_context(nc.semaphore(name="s_pe"))
    s_act = st.enter_context(nc.semaphore(name="s_act"))
    s_dve = st.enter_context(nc.semaphore(name="s_dve"))
    s_out = st.enter_context(nc.semaphore(name="s_out"))

    blk = st.enter_context(nc.Block())

    SLOT = 2 * BSH           # 32 cols per step tile
    OSLOT = OBLK * SLOT      # 1024 cols per staging slot

    def ps_step_ap(ps, set_, sl):
        # (128, 2, 16): the two j-chunk banks of this parity at slice sl
        return ps.rearrange("p (c n) -> p c n", c=4)[
            :, 2 * set_ : 2 * set_ + 2, sl * BSH : (sl + 1) * BSH
        ]

    @blk.sync
    def _(sync):
        sync.dma_start(xext[:], xext_d[:]).then_inc(s_in, 16)
        sync.dma_start(wsb[:], wsb_d[:]).then_inc(s_in, 16)
        sync.dma_start(wb[:], wb_d[:]).then_inc(s_in, 16)
        sync.dma_start(g0[:], g0_d[:]).then_inc(s_in, 16)
        for ob in range(NOBLK):
            ins = sync.dma_start(
                out_d[ob], stage[:, (ob % 2) * OSLOT : (ob % 2 + 1) * OSLOT]
            )
            ins._wait_ge(s_dve, OBLK * (ob + 1))
            ins.then_inc(s_out, 16)
        sync.wait_ge(s_out, 16 * NOBLK)

    @blk.tensor
    def _(pe):
        pe.wait_ge(s_in, 64)
        for t in range(SEQ):
            b, l = divmod(t, BLK)
            set_, sl = l % 2, l // 2
            ps = psA if b % 2 == 0 else psB
            if l == 0:
                # pre-seed the 4 banks of this block with x_t*w_in + b_h
                for sx in range(2):
                    rhs = xext[:, b * 1024 + sx * 512 : b * 1024 + (sx + 1) * 512]
                    for jc in range(2):
                        nc.tensor.matmul(
                            ps[:, (2 * sx + jc) * 512 : (2 * sx + jc + 1) * 512],
                            wb[:, jc * 128 : (jc + 1) * 128],
                            rhs,
                            start=True,
                            stop=False,
                            skip_group_check=True,
                        )
            if t >= 2:
                # psum banks of this parity were read by dve(t-2); its inc
                # makes s_dve = t-1.
                pe.wait_ge(s_dve, t - 1)
            prev = (
                g0[:, :]
                if t == 0
                else gT[:, ((t - 1) % 2) * SLOT : ((t - 1) % 2 + 1) * SLOT]
            )
            first = True
            for jc in range(2):
                for kc in range(2):
                    ins = nc.tensor.matmul(
                        ps[
                            :,
                            set_ * 1024 + jc * 512 + sl * BSH : set_ * 1024
                            + jc * 512
                            + (sl + 1) * BSH,
                        ],
                        wsb[:, (kc * 2 + jc) * 128 : (kc * 2 + jc + 1) * 128],
                        prev[:, kc * BSH : (kc + 1) * BSH],
                        start=False,
                        stop=(kc == 1),
                        skip_group_check=True,
                    )
                    if first:
                        if t > 0:
                            ins._wait_ge(s_act, t)
                        first = False
                    if jc == 1 and kc == 1:
                        ins.then_inc(s_pe, 1)

    @blk.scalar
    def _(act):
        for t in range(SEQ):
            b, l = divmod(t, BLK)
            set_, sl = l % 2, l // 2
            ps = psA if b % 2 == 0 else psB
            out_ap = gT[:, (t % 2) * SLOT : (t % 2 + 1) * SLOT].rearrange(
                "p (c n) -> p c n", c=2
            )
            ins = nc.scalar.activation(out_ap, ps_step_ap(ps, set_, sl), Tanh, scale=0.5)
            ins._wait_ge(s_pe, t + 1)
            ins.then_inc(s_act, 1)

    @blk.vector
    def _(dve):
        for t in range(SEQ):
            b, l = divmod(t, BLK)
            set_, sl = l % 2, l // 2
            ps = psA if b % 2 == 0 else psB
            ob, lo = divmod(t, OBLK)
            if lo == 0 and ob >= 2:
                # staging slot reused; wait for its previous DMA to finish
                dve.wait_ge(s_out, 16 * (ob - 1))
            out_ap = stage[
                :, (ob % 2) * OSLOT + lo * SLOT : (ob % 2) * OSLOT + (lo + 1) * SLOT
            ].rearrange("p (c n) -> p c n", c=2)
            ins = nc.vector.tensor_copy(out_ap, ps_step_ap(ps, set_, sl))
            ins._wait_ge(s_act, t + 1)
            ins.then_inc(s_dve, 1)

    return nc, st


def _get_program():
    if "nc" not in _CACHE:
        nc, st = _build_program()
        _CACHE["nc"] = nc
        _CACHE["st"] = st
    return _CACHE["nc"]


def _bf16(a):
    import ml_dtypes

    return a.astype(ml_dtypes.bfloat16)


def _make_core_inputs(x, h0, W_eff, w_in, b_h):
    """Per-core input dicts. x (SEQ, BATCH) f32; h0 (BATCH, HID) f32."""
    # shared weights
    # wsb[p, (kc*2+jc)*128 + j] = W_eff[jc*128+j, kc*128+p]
    wsb = np.empty((128, 512), np.float32)
    for kc in range(2):
        for jc in range(2):
            blk = W_eff[jc * 128 : (jc + 1) * 128, kc * 128 : (kc + 1) * 128]
            wsb[:, (kc * 2 + jc) * 128 : (kc * 2 + jc + 1) * 128] = blk.T
    wsb = _bf16(wsb)
    wb = np.stack([w_in, b_h]).astype(np.float32)  # (2, 256)

    in_maps = []
    for c in range(NCORES):
        sh = slice(c * BSH, (c + 1) * BSH)
        x_sh = x[:, sh]                                   # (SEQ, 16)
        # xext row0: index = b*1024 + set*512 + e*16 + bi ; t = 64b + 2e + set
        xr = (
            x_sh.reshape(NBLK, BLK // 2, 2, BSH)          # [b, e, set, bi]
            .transpose(0, 2, 1, 3)                        # [b, set, e, bi]
            .reshape(-1)
        )
        xext = np.stack([xr, np.ones_like(xr)]).astype(np.float32)
        g0f = (2.0 * h0[sh, :] - 1.0).astype(np.float32)  # (16, 256)
        # g0[p, c2*16+bi] = g0f[bi, c2*128+p]
        g0 = np.empty((128, 2 * BSH), np.float32)
        for c2 in range(2):
            g0[:, c2 * BSH : (c2 + 1) * BSH] = g0f[:, c2 * 128 : (c2 + 1) * 128].T
        in_maps.append(
            {"xext": xext, "wsb": wsb, "wb": wb, "g0": _bf16(g0)}
        )
    return in_maps


def _run(inputs, trace=False):
    from concourse.bass_utils import run_bass_kernel_spmd

    x = np.asarray(inputs["x"], np.float32)
    h0 = np.asarray(inputs["h0"], np.float32)
    ctx = float(np.asarray(inputs["context"]))
    W_eff = (
        np.asarray(inputs["W_hh"], np.float64)
        + ctx * np.asarray(inputs["W_hh_bias"], np.float64)
    ).astype(np.float32)
    w_in = np.asarray(inputs["W_ih"], np.float32)[:, 0]
    b_h = np.asarray(inputs["b_h"], np.float32)
    W = np.asarray(inputs["W"], np.float64)
    b = np.asarray(inputs["b"], np.float64)

    nc = _get_program()
    in_maps = _make_core_inputs(x, h0, W_eff, w_in, b_h)
    res = run_bass_kernel_spmd(nc, in_maps, list(range(NCORES)), trace=trace)

    # assemble: pre_out (NOBLK, 128, 1024) per core
    pre = np.empty((BATCH, SEQ, HID), np.float32)
    for c in range(NCORES):
        po = res.results[c]["pre_out"]
        # [ob, p, lo*32 + c2*16 + bi] -> [bi, t=ob*32+lo, hid=c2*128+p]
        po = po.reshape(NOBLK, 128, OBLK, 2, BSH).transpose(4, 0, 2, 3, 1)
        pre[c * BSH : (c + 1) * BSH] = po.reshape(BSH, SEQ, HID)

    pre64 = pre.astype(np.float64)
    out64 = 1.0 / (1.0 + np.exp(-pre64))
    y0 = out64 @ W[0, :] + b[0]
    return (y0.astype(np.float32), out64.astype(np.float32)), res


def kernel(**inputs):
    outs, _ = _run(inputs)
    return outs
